# revision 1
# baseline (speedup 1.0000x reference)
"""Trainium2 kernel for nn_ACVRPModel (MatNet-style mixed-score attention).

Strategy: pure data-parallel over batch B=8 across the 8 NeuronCores
(every GEMM / attention / instance-norm / decoder op is batch-local, so
no collectives are needed).  Each core runs the full L=5 encoder +
pointer-decoder for one batch element; outputs are stacked back to the
full (8, 256, 256) tensor.

Self-contained: all shapes/constants hardcoded; no sibling imports.
"""

import numpy as np
import jax
import jax.numpy as jnp

# Problem constants (hardcoded per spec)
B, N, P = 8, 256, 256
D, H, DK, FF, MS, L = 256, 16, 16, 512, 16, 5
SQRT_DK, SQRT_D, CLIP, EPS = 4.0, 16.0, 10.0, 1e-5

_PARAM_NAMES = (
    'enc_Wq', 'enc_Wk', 'enc_Wv', 'mix1_w', 'mix1_b', 'mix2_w', 'mix2_b',
    'comb_w', 'comb_b', 'n1_s', 'n1_b', 'ff_w1', 'ff_b1', 'ff_w2', 'ff_b2',
    'n2_s', 'n2_b', 'dWq0', 'dWq1', 'dWk', 'dWv', 'dcw', 'dcb',
)


def _heads(x):
    b, n, _ = x.shape
    return x.reshape(b, n, H, DK).transpose(0, 2, 1, 3)  # (b,H,n,DK)


def _inorm(x, scale, bias):
    m = x.mean(axis=1, keepdims=True)
    v = x.var(axis=1, keepdims=True)
    return (x - m) * jax.lax.rsqrt(v + EPS) * scale + bias


def _block(row, col, prob, Wq, Wk, Wv, m1w, m1b, m2w, m2b, cw, cb,
           n1s, n1b, w1, b1, w2, b2, n2s, n2b):
    q = _heads(row @ Wq)
    k = _heads(col @ Wk)
    v = _heads(col @ Wv)
    score = jnp.einsum('bhnd,bhmd->bhnm', q, k) / SQRT_DK
    two = jnp.stack(
        [score, jnp.broadcast_to(prob[:, None, :, :], score.shape)], axis=-1)
    hid = jax.nn.relu(jnp.einsum('bhnmi,hio->bhnmo', two, m1w)
                      + m1b[None, :, None, None, :])
    mixed = jnp.einsum('bhnmo,ho->bhnm', hid, m2w) + m2b[None, :, None, None]
    w = jax.nn.softmax(mixed, axis=-1)
    out = jnp.einsum('bhnm,bhmd->bhnd', w, v).transpose(0, 2, 1, 3)
    out = out.reshape(row.shape[0], row.shape[1], H * DK)
    mh = out @ cw + cb
    h1 = _inorm(row + mh, n1s, n1b)
    h2 = jax.nn.relu(h1 @ w1 + b1) @ w2 + b2
    return _inorm(h1 + h2, n2s, n2b)


def _forward(problem, col_emb, mask, current_node, first_node, params):
    """Per-shard forward; batch dim is 1 on each core."""
    (enc_Wq, enc_Wk, enc_Wv, mix1_w, mix1_b, mix2_w, mix2_b,
     comb_w, comb_b, n1_s, n1_b, ff_w1, ff_b1, ff_w2, ff_b2,
     n2_s, n2_b, dWq0, dWq1, dWk, dWv, dcw, dcb) = params
    nb = problem.shape[0]
    row = jnp.zeros_like(col_emb)
    col = col_emb
    plist = (enc_Wq, enc_Wk, enc_Wv, mix1_w, mix1_b, mix2_w, mix2_b,
             comb_w, comb_b, n1_s, n1_b, ff_w1, ff_b1, ff_w2, ff_b2,
             n2_s, n2_b)
    probT = problem.transpose(0, 2, 1)
    for l in range(L):
        pr = [p[l, 0] for p in plist]
        pc = [p[l, 1] for p in plist]
        new_row = _block(row, col, problem, *pr)
        new_col = _block(col, row, probT, *pc)
        row, col = new_row, new_col
    bidx = jnp.arange(nb)[:, None]
    cur = row[bidx, current_node]
    first = row[bidx, first_node]
    q = _heads(cur @ dWq0) + _heads(first @ dWq1)
    k = _heads(col @ dWk)
    v = _heads(col @ dWv)
    sc = jnp.einsum('bhpd,bhmd->bhpm', q, k) / SQRT_DK + mask[:, None, :, :]
    w = jax.nn.softmax(sc, axis=-1)
    out = jnp.einsum('bhpm,bhmd->bhpd', w, v).transpose(0, 2, 1, 3)
    out = out.reshape(nb, P, H * DK)
    sv = out @ dcw + dcb
    logits = jnp.einsum('bpd,bmd->bpm', sv, col) / SQRT_D
    logits = CLIP * jnp.tanh(logits) + mask
    return jax.nn.softmax(logits, axis=2)


_PMAPPED = None


def _get_pmapped():
    global _PMAPPED
    if _PMAPPED is None:
        _PMAPPED = jax.pmap(
            _forward,
            in_axes=(0, 0, 0, 0, 0, None),
            devices=jax.devices()[:8],
        )
    return _PMAPPED


def kernel(**inputs):
    problem = np.asarray(inputs['problem'], dtype=np.float32)
    col_emb = np.asarray(inputs['col_emb'], dtype=np.float32)
    mask = np.asarray(inputs['mask'], dtype=np.float32)
    current_node = np.asarray(inputs['current_node'], dtype=np.int32)
    first_node = np.asarray(inputs['first_node'], dtype=np.int32)
    params = tuple(np.asarray(inputs[n], dtype=np.float32)
                   for n in _PARAM_NAMES)

    # Shard batch across the 8 cores: (8, ...) -> 8 shards of batch 1.
    fn = _get_pmapped()
    out = fn(
        problem.reshape(B, 1, N, N),
        col_emb.reshape(B, 1, N, D),
        mask.reshape(B, 1, P, N),
        current_node.reshape(B, 1, P),
        first_node.reshape(B, 1, P),
        params,
    )
    return np.asarray(out).reshape(B, P, N).astype(np.float32)
